# revision 8
# baseline (speedup 1.0000x reference)
"""Trainium2 Bass kernel for nn_PreCorrectorMLP_StaticDiag.

Reference semantics (see problem):
    idx      = first (E - N) indices where senders != receivers
    gathered = edges_init[idx]
    norm     = max(|gathered|)
    y        = MLP_{1->8->8->1}(gathered / norm)      (relu, pointwise)
    out      = edges_init; out[idx] += alpha * y * norm

With the StaticDiag layout (first N edges are the diagonal, the rest are
guaranteed off-diagonal) idx == arange(N, E).  With zero biases (the oracle
initialises b1 = b2 = b3 = 0) the MLP is positively homogeneous, so
y * norm == a_pos * v  for v >= 0  and  a_neg * v  for v < 0, where the two
slopes come from tiny 8-vector contractions of the weights.  The whole
problem then collapses to one elementwise two-slope multiply over the 14M
off-diagonal tail, which is what the fast device kernel computes:

    out = cneg * v + (cpos - cneg) * max(v, 0)

Sharding: the tail is split evenly across the 8 NeuronCores (data parallel
over edges); no cross-core communication is needed on the fast path because
the norm cancels algebraically.  A general device path (full MLP with
norm / biases baked in as immediates) plus host fallbacks cover every other
case so the kernel is correct for arbitrary inputs of the expected shapes.
"""

import numpy as np

N_NODES = 2_000_000
N_EDGES = 16_000_000
N_TAIL = N_EDGES - N_NODES  # 14_000_000
N_CORES = 8
COLS = 13672                 # per-core free-dim columns (128 partitions)
PER_CORE = 128 * COLS        # 1_750_016 elements per core (128 pad total)
TILE_W = 3418                # 4 tiles of (128, TILE_W) per core
MLP_W = 1709                 # 8 tiles per core on the general MLP path

_NEFF_CACHE: dict = {}
LAST_PATH = None  # debugging breadcrumb: which path the last kernel() call took


# ---------------------------------------------------------------------------
# device kernels
# ---------------------------------------------------------------------------

def _build_twoslope(cneg: float, d: float):
    """out = cneg * v + d * max(v, 0), elementwise over (128, COLS).

    Raw-bass pipeline: all 4 tiles resident in SBUF (no buffer reuse),
    loads issued up-front on the SP HWDGE ring, compute on the vector
    engine, stores on the scalar engine's HWDGE ring so the read and
    write streams drain concurrently.  Per-tile load semaphores are
    required: with several in-flight DMAs on one semaphore the 16
    per-SDMA-engine increments interleave across transfers, so a
    `>= 16*(i+1)` wait would not guarantee tile i has landed."""
    import concourse.bacc as bacc
    import concourse.bass as bass
    import concourse.mybir as mybir

    nt = COLS // TILE_W
    nc = bacc.Bacc("TRN2", target_bir_lowering=False, debug=False,
                   num_devices=N_CORES)
    x = nc.dram_tensor("x", [128, COLS], mybir.dt.float32,
                       kind="ExternalInput").ap()
    y = nc.dram_tensor("y", [128, COLS], mybir.dt.float32,
                       kind="ExternalOutput").ap()
    vs = [nc.alloc_sbuf_tensor(f"v{i}", [128, TILE_W], mybir.dt.float32)
          for i in range(nt)]
    ts_ = [nc.alloc_sbuf_tensor(f"t{i}", [128, TILE_W], mybir.dt.float32)
           for i in range(nt)]
    load_sems = [nc.alloc_semaphore(f"load_sem{i}") for i in range(nt)]
    pipe_sem = nc.alloc_semaphore("pipe_sem")
    dve_sem = nc.alloc_semaphore("dve_sem")
    store_sem = nc.alloc_semaphore("store_sem")

    with nc.Block(no_gpsimd_drain=True) as block:

        @block.sync
        def _(sync):
            for i in range(nt):
                sync.dma_start(vs[i][:], x[:, bass.ts(i, TILE_W)]) \
                    .then_inc(load_sems[i], 16)
            sync.wait_ge(store_sem, 16 * nt)

        @block.vector
        def _(vector):
            for i in range(nt):
                vector.wait_ge(load_sems[i], 16)
                nc.vector.tensor_scalar(ts_[i][:], vs[i][:], 0.0, d,
                                        op0=mybir.AluOpType.max,
                                        op1=mybir.AluOpType.mult) \
                    .then_inc(pipe_sem, 1)
                vector.wait_ge(pipe_sem, i + 1)
                nc.vector.scalar_tensor_tensor(
                    ts_[i][:], vs[i][:], cneg, ts_[i][:],
                    op0=mybir.AluOpType.mult,
                    op1=mybir.AluOpType.add).then_inc(dve_sem, 1)

        @block.scalar
        def _(scalar):
            for i in range(nt):
                scalar.wait_ge(dve_sem, i + 1)
                scalar.dma_start(y[:, bass.ts(i, TILE_W)], ts_[i][:]) \
                    .then_inc(store_sem, 16)

    nc.compile()
    return nc


def _build_mlp(W1, b1, W2, b2, W3, b3, alpha: float, norm: float):
    """General path: out = v + alpha*norm*MLP(v/norm), weights as immediates."""
    import concourse.bacc as bacc
    import concourse.bass as bass
    import concourse.tile as tile
    import concourse.mybir as mybir

    H = 8
    relu = mybir.ActivationFunctionType.Relu
    mult = mybir.AluOpType.mult
    add = mybir.AluOpType.add

    nc = bacc.Bacc("TRN2", target_bir_lowering=False, debug=False,
                   num_devices=N_CORES)
    x = nc.dram_tensor("x", [128, COLS], mybir.dt.float32,
                       kind="ExternalInput").ap()
    yo = nc.dram_tensor("y", [128, COLS], mybir.dt.float32,
                        kind="ExternalOutput").ap()
    with tile.TileContext(nc) as tc:
        with tc.tile_pool(name="vin", bufs=2) as vin, \
             tc.tile_pool(name="h1", bufs=1) as h1p, \
             tc.tile_pool(name="h2", bufs=2) as h2p, \
             tc.tile_pool(name="acc", bufs=2) as accp:
            for i in range(COLS // MLP_W):
                v = vin.tile([128, MLP_W], mybir.dt.float32)
                nc.sync.dma_start(v[:], x[:, bass.ts(i, MLP_W)])
                h1 = h1p.tile([128, H * MLP_W], mybir.dt.float32)
                for c in range(H):
                    hc1 = h1[:, bass.ts(c, MLP_W)]
                    # hc1 = relu((W1[c]/norm) * v + b1[c]); bias folded into
                    # the DVE op (arbitrary float biases have no const AP)
                    nc.vector.tensor_scalar(hc1, v[:],
                                            float(W1[c, 0]) / norm,
                                            float(b1[c]), op0=mult, op1=add)
                    nc.scalar.activation(hc1, hc1, relu, bias=0.0)
                yb = accp.tile([128, MLP_W], mybir.dt.float32)
                for c in range(H):
                    hc = h2p.tile([128, MLP_W], mybir.dt.float32, tag="h2c")
                    nc.vector.tensor_scalar(hc[:], h1[:, bass.ts(0, MLP_W)],
                                            float(W2[c, 0]), float(b2[c]),
                                            op0=mult, op1=add)
                    for k in range(1, H):
                        nc.vector.scalar_tensor_tensor(
                            hc[:], h1[:, bass.ts(k, MLP_W)], float(W2[c, k]),
                            hc[:], op0=mult, op1=add)
                    nc.scalar.activation(hc[:], hc[:], relu, bias=0.0)
                    if c == 0:
                        nc.vector.tensor_scalar(yb[:], hc[:], float(W3[0, 0]),
                                                float(b3[0]), op0=mult,
                                                op1=add)
                    else:
                        nc.vector.scalar_tensor_tensor(yb[:], hc[:],
                                                       float(W3[0, c]), yb[:],
                                                       op0=mult, op1=add)
                # out = (yb * alpha * norm) + v
                nc.vector.scalar_tensor_tensor(yb[:], yb[:], alpha * norm,
                                               v[:], op0=mult, op1=add)
                nc.sync.dma_start(yo[:, bass.ts(i, MLP_W)], yb[:])
    nc.compile()
    return nc


def _run_device(nc, tail: np.ndarray) -> np.ndarray:
    """Shard `tail` (N_TAIL f32) over the 8 cores, run `nc`, gather."""
    from concourse import bass_utils

    padded = np.zeros(N_CORES * PER_CORE, dtype=np.float32)
    padded[:tail.size] = tail
    shards = padded.reshape(N_CORES, 128, COLS)
    in_maps = [{"x": shards[k]} for k in range(N_CORES)]
    last_exc = None
    for _attempt in range(2):  # retry once on transient device errors
        try:
            res = bass_utils.run_bass_kernel_spmd(
                nc, in_maps, core_ids=list(range(N_CORES)))
            break
        except Exception as e:
            last_exc = e
    else:
        raise last_exc
    out = np.concatenate([res.results[k]["y"].reshape(-1)
                          for k in range(N_CORES)])
    return out[:tail.size]


# ---------------------------------------------------------------------------
# host reference pieces (checks + fallbacks)
# ---------------------------------------------------------------------------

def _mlp_numpy(x, W1, b1, W2, b2, W3, b3):
    h = np.maximum(W1.astype(np.float32) @ x[None, :] + b1[:, None], 0.0)
    h = np.maximum(W2.astype(np.float32) @ h + b2[:, None], 0.0)
    return (W3.astype(np.float32) @ h + b3[:, None])[0]


def _host_full(edges_init, idx, W1, b1, W2, b2, W3, b3, alpha):
    gathered = edges_init[idx] if idx is not None else edges_init[N_NODES:]
    norm = np.abs(gathered).max()
    y = _mlp_numpy((gathered / norm).astype(np.float32), W1, b1, W2, b2, W3, b3)
    corr = (alpha * (y * norm)).astype(np.float32)
    out = edges_init.copy()
    if idx is None:
        out[N_NODES:] += corr
    else:
        np.add.at(out, idx, corr)
    return out


def _two_slope(W1, W2, W3, alpha: float):
    w1 = W1[:, 0].astype(np.float64)
    a_pos = float(W3[0].astype(np.float64)
                  @ np.maximum(W2.astype(np.float64) @ np.maximum(w1, 0), 0))
    a_neg = float(W3[0].astype(np.float64)
                  @ np.minimum(W2.astype(np.float64) @ np.minimum(w1, 0), 0))
    cpos = 1.0 + alpha * a_pos
    cneg = 1.0 + alpha * a_neg
    return cneg, cpos - cneg


# ---------------------------------------------------------------------------
# entry point
# ---------------------------------------------------------------------------

def kernel(nodes, edges_init, senders, receivers,
           W1, b1, W2, b2, W3, b3, alpha) -> np.ndarray:
    nodes = np.asarray(nodes)
    edges_init = np.asarray(edges_init, dtype=np.float32)
    senders = np.asarray(senders)
    receivers = np.asarray(receivers)
    W1 = np.asarray(W1, dtype=np.float32)
    b1 = np.asarray(b1, dtype=np.float32)
    W2 = np.asarray(W2, dtype=np.float32)
    b2 = np.asarray(b2, dtype=np.float32)
    W3 = np.asarray(W3, dtype=np.float32)
    b3 = np.asarray(b3, dtype=np.float32)
    alpha_f = float(np.asarray(alpha))

    expected_shapes = (nodes.shape == (N_NODES,)
                       and edges_init.shape == (N_EDGES,)
                       and senders.shape == (N_EDGES,)
                       and receivers.shape == (N_EDGES,)
                       and W1.shape == (8, 1) and b1.shape == (8,)
                       and W2.shape == (8, 8) and b2.shape == (8,)
                       and W3.shape == (1, 8) and b3.shape == (1,))

    nz = senders != receivers
    static = bool(expected_shapes
                  and not nz[:N_NODES].any() and nz[N_NODES:].all())
    if static:
        idx = None
        gathered = edges_init[N_NODES:]
    else:
        n_nondiag = senders.shape[0] - nodes.shape[0]
        idx = np.nonzero(nz)[0][:n_nondiag]
        if idx.size < n_nondiag:  # jnp.nonzero pads with 0
            idx = np.concatenate(
                [idx, np.zeros(n_nondiag - idx.size, dtype=idx.dtype)])
        gathered = edges_init[idx]

    global LAST_PATH
    norm = float(np.abs(gathered).max())
    if not np.isfinite(norm) or norm == 0.0 or not expected_shapes \
            or gathered.size != N_TAIL:
        # degenerate (NaN / all-zero / unexpected shapes): exact host replica
        LAST_PATH = "host-degenerate"
        return _host_full(edges_init, idx, W1, b1, W2, b2, W3, b3, alpha_f)

    zero_bias = not (b1.any() or b2.any() or b3.any())
    use_fast = False
    if zero_bias:
        cneg, d = _two_slope(W1, W2, W3, alpha_f)
        # self-check the homogeneity reduction on a sample before trusting it
        s = gathered[:4096]
        ref = s + alpha_f * (_mlp_numpy((s / norm).astype(np.float32),
                                        W1, b1, W2, b2, W3, b3) * norm)
        fast = np.float32(cneg) * s + np.float32(d) * np.maximum(s, 0.0)
        tol = 1e-4 * max(1.0, float(np.abs(ref).max()))
        use_fast = bool(np.abs(fast - ref).max() < tol)

    try:
        if use_fast:
            key = ("twoslope", cneg, d)
            if key not in _NEFF_CACHE:
                _NEFF_CACHE[key] = _build_twoslope(cneg, d)
            corrected = _run_device(_NEFF_CACHE[key], gathered)
            LAST_PATH = "device-twoslope"
        else:
            key = ("mlp", W1.tobytes(), b1.tobytes(), W2.tobytes(),
                   b2.tobytes(), W3.tobytes(), b3.tobytes(), alpha_f, norm)
            if key not in _NEFF_CACHE:
                _NEFF_CACHE[key] = _build_mlp(W1, b1, W2, b2, W3, b3,
                                              alpha_f, norm)
            corrected = _run_device(_NEFF_CACHE[key], gathered)
            LAST_PATH = "device-mlp"
    except Exception:
        LAST_PATH = "host-fallback"
        return _host_full(edges_init, idx, W1, b1, W2, b2, W3, b3, alpha_f)

    out = edges_init.copy()
    if idx is None:
        out[N_NODES:] = corrected
    else:
        np.add.at(out, idx, corrected - gathered)
    return out


# revision 10
# speedup vs baseline: 1.2399x; 1.2399x over previous
"""Trainium2 Bass kernel for nn_PreCorrectorMLP_StaticDiag.

Reference semantics (see problem):
    idx      = first (E - N) indices where senders != receivers
    gathered = edges_init[idx]
    norm     = max(|gathered|)
    y        = MLP_{1->8->8->1}(gathered / norm)      (relu, pointwise)
    out      = edges_init; out[idx] += alpha * y * norm

With the StaticDiag layout (first N edges are the diagonal, the rest are
guaranteed off-diagonal) idx == arange(N, E).  With zero biases (the oracle
initialises b1 = b2 = b3 = 0) the MLP is positively homogeneous, so
y * norm == a_pos * v  for v >= 0  and  a_neg * v  for v < 0, where the two
slopes come from tiny 8-vector contractions of the weights.  The whole
problem then collapses to one elementwise two-slope multiply over the 14M
off-diagonal tail, which is what the fast device kernel computes:

    out = cneg * v + (cpos - cneg) * max(v, 0)

Sharding: the tail is split evenly across the 8 NeuronCores (data parallel
over edges); no cross-core communication is needed on the fast path because
the norm cancels algebraically.  A general device path (full MLP with
norm / biases baked in as immediates) plus host fallbacks cover every other
case so the kernel is correct for arbitrary inputs of the expected shapes.
"""

import numpy as np

N_NODES = 2_000_000
N_EDGES = 16_000_000
N_TAIL = N_EDGES - N_NODES  # 14_000_000
N_CORES = 8
COLS = 13672                 # per-core free-dim columns (128 partitions)
PER_CORE = 128 * COLS        # 1_750_016 elements per core (128 pad total)
TILE_W = 3418                # 4 tiles of (128, TILE_W) per core
MLP_W = 1709                 # 8 tiles per core on the general MLP path

_NEFF_CACHE: dict = {}
LAST_PATH = None  # debugging breadcrumb: which path the last kernel() call took


# ---------------------------------------------------------------------------
# device kernels
# ---------------------------------------------------------------------------

def _build_twoslope(cneg: float, d: float):
    """out = cneg * v + d * max(v, 0), elementwise over (128, COLS).

    Raw-bass pipeline: all 4 tiles resident in SBUF (no buffer reuse),
    loads issued up-front on the SP HWDGE ring, compute on the vector
    engine, stores on the scalar engine's HWDGE ring so the read and
    write streams drain concurrently.  Per-tile load semaphores are
    required: with several in-flight DMAs on one semaphore the 16
    per-SDMA-engine increments interleave across transfers, so a
    `>= 16*(i+1)` wait would not guarantee tile i has landed.

    No engine waits for store *completion*.  This is safe by construction
    and saves ~6us: the Block-exit all-engine barrier orders the runtime
    epilogue's semaphore resets after every kernel-side wait has passed
    (the store engine only reaches it after its last dve_sem wait), and
    the runtime's final barrier chain provably stalls until the store
    HWDGE ring drains (verified by delaying the last store ~20us: the
    NEFF makespan stretched to cover it and outputs stayed exact).  The
    epilogue resets then overlap the store stream instead of serializing
    behind the last store's completion receipt."""
    import concourse.bacc as bacc
    import concourse.bass as bass
    import concourse.mybir as mybir

    nt = COLS // TILE_W
    nc = bacc.Bacc("TRN2", target_bir_lowering=False, debug=False,
                   num_devices=N_CORES)
    x = nc.dram_tensor("x", [128, COLS], mybir.dt.float32,
                       kind="ExternalInput").ap()
    y = nc.dram_tensor("y", [128, COLS], mybir.dt.float32,
                       kind="ExternalOutput").ap()
    vs = [nc.alloc_sbuf_tensor(f"v{i}", [128, TILE_W], mybir.dt.float32)
          for i in range(nt)]
    ts_ = [nc.alloc_sbuf_tensor(f"t{i}", [128, TILE_W], mybir.dt.float32)
           for i in range(nt)]
    load_sems = [nc.alloc_semaphore(f"load_sem{i}") for i in range(nt)]
    pipe_sem = nc.alloc_semaphore("pipe_sem")
    dve_sem = nc.alloc_semaphore("dve_sem")
    store_sem = nc.alloc_semaphore("store_sem")

    with nc.Block(no_gpsimd_drain=True) as block:

        @block.sync
        def _(sync):
            for i in range(nt):
                sync.dma_start(vs[i][:], x[:, bass.ts(i, TILE_W)]) \
                    .then_inc(load_sems[i], 16)

        @block.vector
        def _(vector):
            for i in range(nt):
                vector.wait_ge(load_sems[i], 16)
                nc.vector.tensor_scalar(ts_[i][:], vs[i][:], 0.0, d,
                                        op0=mybir.AluOpType.max,
                                        op1=mybir.AluOpType.mult) \
                    .then_inc(pipe_sem, 1)
                vector.wait_ge(pipe_sem, i + 1)
                nc.vector.scalar_tensor_tensor(
                    ts_[i][:], vs[i][:], cneg, ts_[i][:],
                    op0=mybir.AluOpType.mult,
                    op1=mybir.AluOpType.add).then_inc(dve_sem, 1)

        @block.scalar
        def _(scalar):
            for i in range(nt):
                scalar.wait_ge(dve_sem, i + 1)
                scalar.dma_start(y[:, bass.ts(i, TILE_W)], ts_[i][:]) \
                    .then_inc(store_sem, 16)

    nc.compile()
    return nc


def _build_mlp(W1, b1, W2, b2, W3, b3, alpha: float, norm: float):
    """General path: out = v + alpha*norm*MLP(v/norm), weights as immediates."""
    import concourse.bacc as bacc
    import concourse.bass as bass
    import concourse.tile as tile
    import concourse.mybir as mybir

    H = 8
    relu = mybir.ActivationFunctionType.Relu
    mult = mybir.AluOpType.mult
    add = mybir.AluOpType.add

    nc = bacc.Bacc("TRN2", target_bir_lowering=False, debug=False,
                   num_devices=N_CORES)
    x = nc.dram_tensor("x", [128, COLS], mybir.dt.float32,
                       kind="ExternalInput").ap()
    yo = nc.dram_tensor("y", [128, COLS], mybir.dt.float32,
                        kind="ExternalOutput").ap()
    with tile.TileContext(nc) as tc:
        with tc.tile_pool(name="vin", bufs=2) as vin, \
             tc.tile_pool(name="h1", bufs=1) as h1p, \
             tc.tile_pool(name="h2", bufs=2) as h2p, \
             tc.tile_pool(name="acc", bufs=2) as accp:
            for i in range(COLS // MLP_W):
                v = vin.tile([128, MLP_W], mybir.dt.float32)
                nc.sync.dma_start(v[:], x[:, bass.ts(i, MLP_W)])
                h1 = h1p.tile([128, H * MLP_W], mybir.dt.float32)
                for c in range(H):
                    hc1 = h1[:, bass.ts(c, MLP_W)]
                    # hc1 = relu((W1[c]/norm) * v + b1[c]); bias folded into
                    # the DVE op (arbitrary float biases have no const AP)
                    nc.vector.tensor_scalar(hc1, v[:],
                                            float(W1[c, 0]) / norm,
                                            float(b1[c]), op0=mult, op1=add)
                    nc.scalar.activation(hc1, hc1, relu, bias=0.0)
                yb = accp.tile([128, MLP_W], mybir.dt.float32)
                for c in range(H):
                    hc = h2p.tile([128, MLP_W], mybir.dt.float32, tag="h2c")
                    nc.vector.tensor_scalar(hc[:], h1[:, bass.ts(0, MLP_W)],
                                            float(W2[c, 0]), float(b2[c]),
                                            op0=mult, op1=add)
                    for k in range(1, H):
                        nc.vector.scalar_tensor_tensor(
                            hc[:], h1[:, bass.ts(k, MLP_W)], float(W2[c, k]),
                            hc[:], op0=mult, op1=add)
                    nc.scalar.activation(hc[:], hc[:], relu, bias=0.0)
                    if c == 0:
                        nc.vector.tensor_scalar(yb[:], hc[:], float(W3[0, 0]),
                                                float(b3[0]), op0=mult,
                                                op1=add)
                    else:
                        nc.vector.scalar_tensor_tensor(yb[:], hc[:],
                                                       float(W3[0, c]), yb[:],
                                                       op0=mult, op1=add)
                # out = (yb * alpha * norm) + v
                nc.vector.scalar_tensor_tensor(yb[:], yb[:], alpha * norm,
                                               v[:], op0=mult, op1=add)
                nc.sync.dma_start(yo[:, bass.ts(i, MLP_W)], yb[:])
    nc.compile()
    return nc


def _run_device(nc, tail: np.ndarray) -> np.ndarray:
    """Shard `tail` (N_TAIL f32) over the 8 cores, run `nc`, gather."""
    from concourse import bass_utils

    padded = np.zeros(N_CORES * PER_CORE, dtype=np.float32)
    padded[:tail.size] = tail
    shards = padded.reshape(N_CORES, 128, COLS)
    in_maps = [{"x": shards[k]} for k in range(N_CORES)]
    last_exc = None
    for _attempt in range(2):  # retry once on transient device errors
        try:
            res = bass_utils.run_bass_kernel_spmd(
                nc, in_maps, core_ids=list(range(N_CORES)))
            break
        except Exception as e:
            last_exc = e
    else:
        raise last_exc
    out = np.concatenate([res.results[k]["y"].reshape(-1)
                          for k in range(N_CORES)])
    return out[:tail.size]


# ---------------------------------------------------------------------------
# host reference pieces (checks + fallbacks)
# ---------------------------------------------------------------------------

def _mlp_numpy(x, W1, b1, W2, b2, W3, b3):
    h = np.maximum(W1.astype(np.float32) @ x[None, :] + b1[:, None], 0.0)
    h = np.maximum(W2.astype(np.float32) @ h + b2[:, None], 0.0)
    return (W3.astype(np.float32) @ h + b3[:, None])[0]


def _host_full(edges_init, idx, W1, b1, W2, b2, W3, b3, alpha):
    gathered = edges_init[idx] if idx is not None else edges_init[N_NODES:]
    norm = np.abs(gathered).max()
    y = _mlp_numpy((gathered / norm).astype(np.float32), W1, b1, W2, b2, W3, b3)
    corr = (alpha * (y * norm)).astype(np.float32)
    out = edges_init.copy()
    if idx is None:
        out[N_NODES:] += corr
    else:
        np.add.at(out, idx, corr)
    return out


def _two_slope(W1, W2, W3, alpha: float):
    w1 = W1[:, 0].astype(np.float64)
    a_pos = float(W3[0].astype(np.float64)
                  @ np.maximum(W2.astype(np.float64) @ np.maximum(w1, 0), 0))
    a_neg = float(W3[0].astype(np.float64)
                  @ np.minimum(W2.astype(np.float64) @ np.minimum(w1, 0), 0))
    cpos = 1.0 + alpha * a_pos
    cneg = 1.0 + alpha * a_neg
    return cneg, cpos - cneg


# ---------------------------------------------------------------------------
# entry point
# ---------------------------------------------------------------------------

def kernel(nodes, edges_init, senders, receivers,
           W1, b1, W2, b2, W3, b3, alpha) -> np.ndarray:
    nodes = np.asarray(nodes)
    edges_init = np.asarray(edges_init, dtype=np.float32)
    senders = np.asarray(senders)
    receivers = np.asarray(receivers)
    W1 = np.asarray(W1, dtype=np.float32)
    b1 = np.asarray(b1, dtype=np.float32)
    W2 = np.asarray(W2, dtype=np.float32)
    b2 = np.asarray(b2, dtype=np.float32)
    W3 = np.asarray(W3, dtype=np.float32)
    b3 = np.asarray(b3, dtype=np.float32)
    alpha_f = float(np.asarray(alpha))

    expected_shapes = (nodes.shape == (N_NODES,)
                       and edges_init.shape == (N_EDGES,)
                       and senders.shape == (N_EDGES,)
                       and receivers.shape == (N_EDGES,)
                       and W1.shape == (8, 1) and b1.shape == (8,)
                       and W2.shape == (8, 8) and b2.shape == (8,)
                       and W3.shape == (1, 8) and b3.shape == (1,))

    nz = senders != receivers
    static = bool(expected_shapes
                  and not nz[:N_NODES].any() and nz[N_NODES:].all())
    if static:
        idx = None
        gathered = edges_init[N_NODES:]
    else:
        n_nondiag = senders.shape[0] - nodes.shape[0]
        idx = np.nonzero(nz)[0][:n_nondiag]
        if idx.size < n_nondiag:  # jnp.nonzero pads with 0
            idx = np.concatenate(
                [idx, np.zeros(n_nondiag - idx.size, dtype=idx.dtype)])
        gathered = edges_init[idx]

    global LAST_PATH
    norm = float(np.abs(gathered).max())
    if not np.isfinite(norm) or norm == 0.0 or not expected_shapes \
            or gathered.size != N_TAIL:
        # degenerate (NaN / all-zero / unexpected shapes): exact host replica
        LAST_PATH = "host-degenerate"
        return _host_full(edges_init, idx, W1, b1, W2, b2, W3, b3, alpha_f)

    zero_bias = not (b1.any() or b2.any() or b3.any())
    use_fast = False
    if zero_bias:
        cneg, d = _two_slope(W1, W2, W3, alpha_f)
        # self-check the homogeneity reduction on a sample before trusting it
        s = gathered[:4096]
        ref = s + alpha_f * (_mlp_numpy((s / norm).astype(np.float32),
                                        W1, b1, W2, b2, W3, b3) * norm)
        fast = np.float32(cneg) * s + np.float32(d) * np.maximum(s, 0.0)
        tol = 1e-4 * max(1.0, float(np.abs(ref).max()))
        use_fast = bool(np.abs(fast - ref).max() < tol)

    try:
        if use_fast:
            key = ("twoslope", cneg, d)
            if key not in _NEFF_CACHE:
                _NEFF_CACHE[key] = _build_twoslope(cneg, d)
            corrected = _run_device(_NEFF_CACHE[key], gathered)
            LAST_PATH = "device-twoslope"
        else:
            key = ("mlp", W1.tobytes(), b1.tobytes(), W2.tobytes(),
                   b2.tobytes(), W3.tobytes(), b3.tobytes(), alpha_f, norm)
            if key not in _NEFF_CACHE:
                _NEFF_CACHE[key] = _build_mlp(W1, b1, W2, b2, W3, b3,
                                              alpha_f, norm)
            corrected = _run_device(_NEFF_CACHE[key], gathered)
            LAST_PATH = "device-mlp"
    except Exception:
        LAST_PATH = "host-fallback"
        return _host_full(edges_init, idx, W1, b1, W2, b2, W3, b3, alpha_f)

    out = edges_init.copy()
    if idx is None:
        out[N_NODES:] = corrected
    else:
        np.add.at(out, idx, corrected - gathered)
    return out
